# revision 23
# baseline (speedup 1.0000x reference)
"""BinaryDiff kernel for 8 TRN2 NeuronCores.

Computes out = x @ base + coeff * (x @ (2*mask - 1)) for
x [4,2048,4096] f32, base [4096,4096] f32, mask [4096,4096] i32,
coeff [] f32 -> out [4,2048,4096] f32.

Key algebraic fusion: dense + coeff*binary = x @ (base + coeff*(2*mask-1)),
so we fuse the weights on-device (one elementwise pass over base/mask) and
run a SINGLE matmul in bf16 (fp32 PSUM accumulation).

Sharding (tensor-parallel 2x4 grid, no collectives):
  - rows (B*S = 8192) split 2 ways  -> 4096 rows/core
  - out cols (4096)   split 4 ways  -> 1024 cols/core
Each core fuses its own W shard [4096, 1024] (bf16, SBUF-resident),
streams its x shard block-by-block (128 rows), casts to bf16, transposes
k-tiles via the PE array, and accumulates [128, 512] PSUM tiles over K.

Pipeline shape (from trace analysis): the contraction K=4096 is split into
quarter-rounds whose PSUM partials are combined on the DVE; blocks run in
super-blocks of 8, every super-block streams x in round-aligned column
chunks (small live x^T set, transposes interleave between rounds), chunk
production runs one stage ahead of the matmuls, and during super-block 0
the W-fusion DMAs are woven 1:1 with the x chunks of the matching k range.
Measured ~555 us on silicon (PE issue-rate-limited; bf16 matmul roofline
for this shard is ~442 us + ~66 us of PE transposes).
"""

import numpy as np
from contextlib import ExitStack

import concourse.bass as bass
import concourse.mybir as mybir
import concourse.tile as tile
from concourse import bacc
from concourse.bass_utils import run_bass_kernel_spmd
from concourse.masks import make_identity

P = 128
B, S, DIN, DOUT = 4, 2048, 4096, 4096
P_ROWS, Q_COLS = 2, 4           # core grid: 2 row-shards x 4 col-shards
BS = B * S                      # 8192
BS_C = BS // P_ROWS             # 4096 rows per core
NO_C = DOUT // Q_COLS           # 1024 out cols per core
MM_N = 512                      # matmul moving free dim (1 PSUM bank of f32)
SB_G = 8                        # blocks per super-block

f32 = mybir.dt.float32
bf16 = mybir.dt.bfloat16
i32 = mybir.dt.int32


def emit_kernel(tc, x_ap, base_ap, mask_ap, coeff_ap, out_ap,
                bs_c, din, no_c):
    """Emit the per-core Tile program. Shapes parameterized for sim tests."""
    nc = tc.nc
    kt_n = din // P            # k tiles
    nblk = bs_c // P           # 128-row x blocks
    ot_n = no_c // MM_N        # psum-group column tiles per block
    half = din // 2            # x row-block loaded as two DMAs (steady state)
    n_rounds = 4 if kt_n % 4 == 0 else 2
    kq = kt_n // n_rounds      # k tiles per round
    rounds = [(r * kq, (r + 1) * kq) for r in range(n_rounds)]
    sw = min(4, kq)           # k-tiles per x^T strip

    with ExitStack() as ctx:
        const = ctx.enter_context(tc.tile_pool(name="const", bufs=1))
        wpool = ctx.enter_context(tc.tile_pool(name="wpool", bufs=kt_n))
        fb = ctx.enter_context(tc.tile_pool(name="fbase", bufs=3))
        fm = ctx.enter_context(tc.tile_pool(name="fmask", bufs=3))
        fs = ctx.enter_context(tc.tile_pool(name="fsgn", bufs=2))
        xfp = ctx.enter_context(tc.tile_pool(name="xf", bufs=4))
        xbp = ctx.enter_context(tc.tile_pool(name="xb", bufs=5))
        xtp = ctx.enter_context(tc.tile_pool(name="xt", bufs=33))
        evp = ctx.enter_context(tc.tile_pool(name="ev", bufs=SB_G + 1))
        mmp = ctx.enter_context(tc.tile_pool(name="mmpsum", bufs=6, space="PSUM"))
        trp = ctx.enter_context(tc.tile_pool(name="trpsum", bufs=2, space="PSUM"))

        # --- constants: identity for PE transpose, coeff broadcast ---
        ident = const.tile([P, P], bf16)
        make_identity(nc, ident)
        # PE warmup: the HAM clock-gate only lifts to 2.4 GHz after ~3.4us of
        # sustained PE activity. The PE is data-starved for the first ~6us
        # anyway, so burn that window on dummy transposes to warm the clock
        # before the real matmul stream begins.
        for _ in range(20):
            pw = trp.tile([P, P], bf16, tag="pt", name="pt")
            nc.tensor.transpose(pw[:], ident[:], ident[:])

        c_sb = const.tile([1, 1], f32)
        nc.sync.dma_start(c_sb[:], coeff_ap[:])
        ones = const.tile([1, P], f32)
        nc.any.memset(ones[:], 1.0)
        cps = mmp.tile([P, MM_N], f32, tag="ps")
        # [128,1] = ones.T @ coeff : broadcasts the runtime scalar across partitions
        nc.tensor.matmul(cps[:, 0:1], ones[:], c_sb[:], start=True, stop=True)
        twoc = const.tile([P, 1], f32)
        negc = const.tile([P, 1], f32)
        nc.vector.tensor_scalar_mul(twoc[:], cps[:, 0:1], 2.0)
        nc.vector.tensor_scalar_mul(negc[:], cps[:, 0:1], -1.0)

        # --- W fusion: W[kt] = bf16(base + (2c)*mask - c), SBUF resident ---
        wtiles = [None] * kt_n

        def emit_fusion(kt):
            bt = fb.tile([P, no_c], f32)
            nc.sync.dma_start(bt[:], base_ap[kt * P:(kt + 1) * P, :])
            mt = fm.tile([P, no_c], i32)
            nc.sync.dma_start(mt[:], mask_ap[kt * P:(kt + 1) * P, :])
            sg = fs.tile([P, no_c], f32)
            nc.gpsimd.tensor_scalar(sg[:], mt[:], twoc[:], negc[:],
                                    mybir.AluOpType.mult, mybir.AluOpType.add)
            wt = wpool.tile([P, no_c], bf16)
            nc.vector.tensor_tensor(wt[:], sg[:], bt[:], mybir.AluOpType.add)
            wtiles[kt] = wt

        # --- per-block strip production: load, cast, PE-transpose ---
        # chunk = x columns [c0, c1); produces strips (one [128, 4*128]
        # x^T tile per 4 k-tiles) for those columns into `strips` dict.
        def emit_chunk(b, c0, c1, strips):
            row = slice(b * P, (b + 1) * P)
            xf = xfp.tile([P, c1 - c0], f32, tag="xf", name="xf")
            nc.sync.dma_start(xf[:], x_ap[row, c0:c1])
            xb = xbp.tile([P, c1 - c0], bf16, tag="xb", name="xb")
            nc.scalar.copy(xb[:], xf[:])
            for s in range((c1 - c0) // (sw * P)):
                pt = trp.tile([P, sw * P], bf16, tag="pt", name="pt")
                for j in range(sw):
                    col = s * sw * P + j * P
                    nc.tensor.transpose(pt[:, j * P:(j + 1) * P],
                                        xb[:, col:col + P], ident[:])
                st = xtp.tile([P, sw * P], bf16, tag="st", name="st")
                nc.vector.tensor_copy(st[:], pt[:])
                strips[(c0 + s * sw * P) // P] = st

        def emit_round(strips, ev, kt_lo, kt_hi, first):
            for ot in range(ot_n):
                ps = mmp.tile([P, MM_N], f32, tag="ps", name="ps")
                for kt in range(kt_lo, kt_hi):
                    st = strips[(kt // sw) * sw]
                    nc.tensor.matmul(
                        ps[:],
                        st[:, (kt % sw) * P:(kt % sw + 1) * P],
                        wtiles[kt][:, ot * MM_N:(ot + 1) * MM_N],
                        start=(kt == kt_lo), stop=(kt == kt_hi - 1),
                    )
                evs = ev[:, ot * MM_N:(ot + 1) * MM_N]
                if first:
                    nc.scalar.copy(evs, ps[:])
                else:
                    nc.vector.tensor_tensor(evs, evs, ps[:],
                                            mybir.AluOpType.add)

        # --- main schedule ---
        # Unified (block, k-round) staging with one-stage lookahead: the x
        # chunks (DMA + cast + PE-transpose) for stage s+1 are emitted before
        # the matmul rounds of stage s, so strip production overlaps the
        # previous round's matmuls and round boundaries have no PE->DVE->PE
        # handoff bubble. During super-block 0 the W-fusion DMAs are woven
        # 1:1 with the x chunks of the matching k range.
        fused = [False] * kt_n
        strips_of = {}
        ev_of = {}
        # Super-block 0 runs twice-as-fine k-rounds (matching the W-fusion
        # arrival granularity); steady super-blocks use the coarser rounds.
        rounds0 = rounds
        stages = []
        for sb0 in range(0, nblk, SB_G):
            blocks = list(range(sb0, min(sb0 + SB_G, nblk)))
            rl = rounds0 if sb0 == 0 else rounds
            for ri, (klo, khi) in enumerate(rl):
                stages.append((blocks, klo, khi, ri == 0, ri == len(rl) - 1))

        def emit_stage_chunks(blocks, klo, khi):
            kts_todo = [kt for kt in range(klo, khi) if not fused[kt]]
            nb = len(blocks)
            for i, b in enumerate(blocks):
                if b not in ev_of:
                    strips_of[b] = {}
                    ev_of[b] = evp.tile([P, no_c], f32, tag="ev", name="ev")
                emit_chunk(b, klo * P, khi * P, strips_of[b])
                n0 = len(kts_todo) * i // nb
                n1 = len(kts_todo) * (i + 1) // nb
                for kt in kts_todo[n0:n1]:
                    emit_fusion(kt)
                    fused[kt] = True

        emit_stage_chunks(*stages[0][:3])
        for si, (blocks, klo, khi, first, last) in enumerate(stages):
            if si + 1 < len(stages):
                emit_stage_chunks(*stages[si + 1][:3])
            for b in blocks:
                emit_round(strips_of[b], ev_of[b], klo, khi, first=first)
                if last:
                    row = slice(b * P, (b + 1) * P)
                    nc.sync.dma_start(out_ap[row, :], ev_of[b][:])
            if last:
                for b in blocks:
                    del strips_of[b], ev_of[b]


def build_nc(bs_c=BS_C, din=DIN, no_c=NO_C):
    nc = bacc.Bacc("TRN2", target_bir_lowering=False, debug=False, num_devices=8)
    x_ap = nc.dram_tensor("x", [bs_c, din], f32, kind="ExternalInput").ap()
    base_ap = nc.dram_tensor("base", [din, no_c], f32, kind="ExternalInput").ap()
    mask_ap = nc.dram_tensor("mask", [din, no_c], i32, kind="ExternalInput").ap()
    coeff_ap = nc.dram_tensor("coeff", [1, 1], f32, kind="ExternalInput").ap()
    out_ap = nc.dram_tensor("out", [bs_c, no_c], f32, kind="ExternalOutput").ap()
    with tile.TileContext(nc) as tc:
        emit_kernel(tc, x_ap, base_ap, mask_ap, coeff_ap, out_ap,
                    bs_c, din, no_c)
    nc.compile()
    return nc


_NC_CACHE = {}


def _get_nc():
    if "nc" not in _NC_CACHE:
        _NC_CACHE["nc"] = build_nc()
    return _NC_CACHE["nc"]


def make_in_maps(x, base, mask, coeff):
    """Shard full inputs across the 2x4 core grid (cores 0..7)."""
    xf = np.ascontiguousarray(x.reshape(BS, DIN).astype(np.float32, copy=False))
    coeff2d = np.asarray(coeff, dtype=np.float32).reshape(1, 1)
    in_maps = []
    for cid in range(8):
        pi, qi = divmod(cid, Q_COLS)
        in_maps.append({
            "x": np.ascontiguousarray(xf[pi * BS_C:(pi + 1) * BS_C, :]),
            "base": np.ascontiguousarray(
                base[:, qi * NO_C:(qi + 1) * NO_C].astype(np.float32, copy=False)),
            "mask": np.ascontiguousarray(
                mask[:, qi * NO_C:(qi + 1) * NO_C].astype(np.int32, copy=False)),
            "coeff": coeff2d,
        })
    return in_maps


def assemble_out(results):
    out = np.empty((BS, DOUT), dtype=np.float32)
    for cid in range(8):
        pi, qi = divmod(cid, Q_COLS)
        out[pi * BS_C:(pi + 1) * BS_C, qi * NO_C:(qi + 1) * NO_C] = \
            results[cid]["out"]
    return out.reshape(B, S, DOUT)


def kernel(x, base, mask, coeff):
    nc = _get_nc()
    in_maps = make_in_maps(np.asarray(x), np.asarray(base),
                           np.asarray(mask), np.asarray(coeff))
    res = run_bass_kernel_spmd(nc, in_maps, core_ids=list(range(8)))
    return assemble_out(res.results)


# revision 24
# speedup vs baseline: 1.0093x; 1.0093x over previous
"""BinaryDiff kernel for 8 TRN2 NeuronCores.

Computes out = x @ base + coeff * (x @ (2*mask - 1)) for
x [4,2048,4096] f32, base [4096,4096] f32, mask [4096,4096] i32,
coeff [] f32 -> out [4,2048,4096] f32.

Key algebraic fusion: dense + coeff*binary = x @ (base + coeff*(2*mask-1)),
so we fuse the weights on-device (one elementwise pass over base/mask) and
run a SINGLE matmul in bf16 (fp32 PSUM accumulation).

Sharding (tensor-parallel 2x4 grid, no collectives):
  - rows (B*S = 8192) split 2 ways  -> 4096 rows/core
  - out cols (4096)   split 4 ways  -> 1024 cols/core
Each core fuses its own W shard [4096, 1024] (bf16, SBUF-resident),
streams its x shard block-by-block (128 rows), casts to bf16, transposes
k-tiles via the PE array, and accumulates [128, 512] PSUM tiles over K.

Pipeline shape (from trace analysis): the contraction K=4096 is split into
quarter-rounds whose PSUM partials are combined on the DVE; blocks run in
super-blocks of 8, every super-block streams x in round-aligned column
chunks (small live x^T set, transposes interleave between rounds), chunk
production runs one stage ahead of the matmuls, and during super-block 0
the W-fusion DMAs are woven 1:1 with the x chunks of the matching k range.
Measured ~555 us on silicon (PE issue-rate-limited; bf16 matmul roofline
for this shard is ~442 us + ~66 us of PE transposes).
"""

import numpy as np
from contextlib import ExitStack

import concourse.bass as bass
import concourse.mybir as mybir
import concourse.tile as tile
from concourse import bacc
from concourse.bass_utils import run_bass_kernel_spmd
from concourse.masks import make_identity

P = 128
B, S, DIN, DOUT = 4, 2048, 4096, 4096
P_ROWS, Q_COLS = 2, 4           # core grid: 2 row-shards x 4 col-shards
BS = B * S                      # 8192
BS_C = BS // P_ROWS             # 4096 rows per core
NO_C = DOUT // Q_COLS           # 1024 out cols per core
MM_N = 512                      # matmul moving free dim (1 PSUM bank of f32)
SB_G = 8                        # blocks per super-block

f32 = mybir.dt.float32
bf16 = mybir.dt.bfloat16
i32 = mybir.dt.int32


def emit_kernel(tc, x_ap, base_ap, mask_ap, coeff_ap, out_ap,
                bs_c, din, no_c):
    """Emit the per-core Tile program. Shapes parameterized for sim tests."""
    nc = tc.nc
    kt_n = din // P            # k tiles
    nblk = bs_c // P           # 128-row x blocks
    ot_n = no_c // MM_N        # psum-group column tiles per block
    half = din // 2            # x row-block loaded as two DMAs (steady state)
    n_rounds = 4 if kt_n % 4 == 0 else 2
    kq = kt_n // n_rounds      # k tiles per round
    rounds = [(r * kq, (r + 1) * kq) for r in range(n_rounds)]
    sw = min(4, kq)           # k-tiles per x^T strip

    with ExitStack() as ctx:
        const = ctx.enter_context(tc.tile_pool(name="const", bufs=1))
        wpool = ctx.enter_context(tc.tile_pool(name="wpool", bufs=kt_n))
        fb = ctx.enter_context(tc.tile_pool(name="fbase", bufs=3))
        fm = ctx.enter_context(tc.tile_pool(name="fmask", bufs=3))
        fs = ctx.enter_context(tc.tile_pool(name="fsgn", bufs=2))
        xfp = ctx.enter_context(tc.tile_pool(name="xf", bufs=4))
        xbp = ctx.enter_context(tc.tile_pool(name="xb", bufs=5))
        xtp = ctx.enter_context(tc.tile_pool(name="xt", bufs=33))
        evp = ctx.enter_context(tc.tile_pool(name="ev", bufs=SB_G + 1))
        mmp = ctx.enter_context(tc.tile_pool(name="mmpsum", bufs=6, space="PSUM"))
        trp = ctx.enter_context(tc.tile_pool(name="trpsum", bufs=2, space="PSUM"))

        # --- constants: identity for PE transpose, coeff broadcast ---
        ident = const.tile([P, P], bf16)
        make_identity(nc, ident)
        # PE warmup: the HAM clock-gate only lifts to 2.4 GHz after ~3.4us of
        # sustained PE activity. The PE is data-starved for the first ~6us
        # anyway, so burn that window on dummy transposes to warm the clock
        # before the real matmul stream begins.
        for _ in range(20):
            pw = trp.tile([P, P], bf16, tag="pt", name="pt")
            nc.tensor.transpose(pw[:], ident[:], ident[:])

        c_sb = const.tile([1, 1], f32)
        nc.sync.dma_start(c_sb[:], coeff_ap[:])
        ones = const.tile([1, P], f32)
        nc.any.memset(ones[:], 1.0)
        cps = mmp.tile([P, MM_N], f32, tag="ps")
        # [128,1] = ones.T @ coeff : broadcasts the runtime scalar across partitions
        nc.tensor.matmul(cps[:, 0:1], ones[:], c_sb[:], start=True, stop=True)
        twoc = const.tile([P, 1], f32)
        negc = const.tile([P, 1], f32)
        nc.vector.tensor_scalar_mul(twoc[:], cps[:, 0:1], 2.0)
        nc.vector.tensor_scalar_mul(negc[:], cps[:, 0:1], -1.0)

        # --- W fusion: W[kt] = bf16(base + (2c)*mask - c), SBUF resident ---
        wtiles = [None] * kt_n

        def emit_fusion(kt):
            bt = fb.tile([P, no_c], f32)
            nc.sync.dma_start(bt[:], base_ap[kt * P:(kt + 1) * P, :])
            mt = fm.tile([P, no_c], i32)
            nc.sync.dma_start(mt[:], mask_ap[kt * P:(kt + 1) * P, :])
            sg = fs.tile([P, no_c], f32)
            nc.gpsimd.tensor_scalar(sg[:], mt[:], twoc[:], negc[:],
                                    mybir.AluOpType.mult, mybir.AluOpType.add)
            wt = wpool.tile([P, no_c], bf16)
            nc.vector.tensor_tensor(wt[:], sg[:], bt[:], mybir.AluOpType.add)
            wtiles[kt] = wt

        # --- per-block strip production: load, cast, PE-transpose ---
        # chunk = x columns [c0, c1); produces strips (one [128, 4*128]
        # x^T tile per 4 k-tiles) for those columns into `strips` dict.
        def emit_chunk(b, c0, c1, strips):
            row = slice(b * P, (b + 1) * P)
            xf = xfp.tile([P, c1 - c0], f32, tag="xf", name="xf")
            nc.sync.dma_start(xf[:], x_ap[row, c0:c1])
            xb = xbp.tile([P, c1 - c0], bf16, tag="xb", name="xb")
            nc.scalar.copy(xb[:], xf[:])
            for s in range((c1 - c0) // (sw * P)):
                pt = trp.tile([P, sw * P], bf16, tag="pt", name="pt")
                for j in range(sw):
                    col = s * sw * P + j * P
                    nc.tensor.transpose(pt[:, j * P:(j + 1) * P],
                                        xb[:, col:col + P], ident[:])
                st = xtp.tile([P, sw * P], bf16, tag="st", name="st")
                nc.vector.tensor_copy(st[:], pt[:])
                strips[(c0 + s * sw * P) // P] = st

        def emit_round(strips, ev, kt_lo, kt_hi, first):
            for ot in range(ot_n):
                ps = mmp.tile([P, MM_N], f32, tag="ps", name="ps")
                for kt in range(kt_lo, kt_hi):
                    st = strips[(kt // sw) * sw]
                    nc.tensor.matmul(
                        ps[:],
                        st[:, (kt % sw) * P:(kt % sw + 1) * P],
                        wtiles[kt][:, ot * MM_N:(ot + 1) * MM_N],
                        start=(kt == kt_lo), stop=(kt == kt_hi - 1),
                    )
                evs = ev[:, ot * MM_N:(ot + 1) * MM_N]
                if first:
                    nc.vector.tensor_copy(evs, ps[:])
                else:
                    nc.vector.tensor_tensor(evs, evs, ps[:],
                                            mybir.AluOpType.add)

        # --- main schedule ---
        # Unified (block, k-round) staging with one-stage lookahead: the x
        # chunks (DMA + cast + PE-transpose) for stage s+1 are emitted before
        # the matmul rounds of stage s, so strip production overlaps the
        # previous round's matmuls and round boundaries have no PE->DVE->PE
        # handoff bubble. During super-block 0 the W-fusion DMAs are woven
        # 1:1 with the x chunks of the matching k range.
        fused = [False] * kt_n
        strips_of = {}
        ev_of = {}
        # Super-block 0 runs twice-as-fine k-rounds (matching the W-fusion
        # arrival granularity); steady super-blocks use the coarser rounds.
        rounds0 = rounds
        stages = []
        for sb0 in range(0, nblk, SB_G):
            blocks = list(range(sb0, min(sb0 + SB_G, nblk)))
            rl = rounds0 if sb0 == 0 else rounds
            for ri, (klo, khi) in enumerate(rl):
                stages.append((blocks, klo, khi, ri == 0, ri == len(rl) - 1))

        def emit_stage_chunks(blocks, klo, khi):
            kts_todo = [kt for kt in range(klo, khi) if not fused[kt]]
            nb = len(blocks)
            for i, b in enumerate(blocks):
                if b not in ev_of:
                    strips_of[b] = {}
                    ev_of[b] = evp.tile([P, no_c], f32, tag="ev", name="ev")
                emit_chunk(b, klo * P, khi * P, strips_of[b])
                n0 = len(kts_todo) * i // nb
                n1 = len(kts_todo) * (i + 1) // nb
                for kt in kts_todo[n0:n1]:
                    emit_fusion(kt)
                    fused[kt] = True

        emit_stage_chunks(*stages[0][:3])
        for si, (blocks, klo, khi, first, last) in enumerate(stages):
            if si + 1 < len(stages):
                emit_stage_chunks(*stages[si + 1][:3])
            for b in blocks:
                emit_round(strips_of[b], ev_of[b], klo, khi, first=first)
                if last:
                    row = slice(b * P, (b + 1) * P)
                    nc.sync.dma_start(out_ap[row, :], ev_of[b][:])
            if last:
                for b in blocks:
                    del strips_of[b], ev_of[b]


def build_nc(bs_c=BS_C, din=DIN, no_c=NO_C):
    nc = bacc.Bacc("TRN2", target_bir_lowering=False, debug=False, num_devices=8)
    x_ap = nc.dram_tensor("x", [bs_c, din], f32, kind="ExternalInput").ap()
    base_ap = nc.dram_tensor("base", [din, no_c], f32, kind="ExternalInput").ap()
    mask_ap = nc.dram_tensor("mask", [din, no_c], i32, kind="ExternalInput").ap()
    coeff_ap = nc.dram_tensor("coeff", [1, 1], f32, kind="ExternalInput").ap()
    out_ap = nc.dram_tensor("out", [bs_c, no_c], f32, kind="ExternalOutput").ap()
    with tile.TileContext(nc) as tc:
        emit_kernel(tc, x_ap, base_ap, mask_ap, coeff_ap, out_ap,
                    bs_c, din, no_c)
    nc.compile()
    return nc


_NC_CACHE = {}


def _get_nc():
    if "nc" not in _NC_CACHE:
        _NC_CACHE["nc"] = build_nc()
    return _NC_CACHE["nc"]


def make_in_maps(x, base, mask, coeff):
    """Shard full inputs across the 2x4 core grid (cores 0..7)."""
    xf = np.ascontiguousarray(x.reshape(BS, DIN).astype(np.float32, copy=False))
    coeff2d = np.asarray(coeff, dtype=np.float32).reshape(1, 1)
    in_maps = []
    for cid in range(8):
        pi, qi = divmod(cid, Q_COLS)
        in_maps.append({
            "x": np.ascontiguousarray(xf[pi * BS_C:(pi + 1) * BS_C, :]),
            "base": np.ascontiguousarray(
                base[:, qi * NO_C:(qi + 1) * NO_C].astype(np.float32, copy=False)),
            "mask": np.ascontiguousarray(
                mask[:, qi * NO_C:(qi + 1) * NO_C].astype(np.int32, copy=False)),
            "coeff": coeff2d,
        })
    return in_maps


def assemble_out(results):
    out = np.empty((BS, DOUT), dtype=np.float32)
    for cid in range(8):
        pi, qi = divmod(cid, Q_COLS)
        out[pi * BS_C:(pi + 1) * BS_C, qi * NO_C:(qi + 1) * NO_C] = \
            results[cid]["out"]
    return out.reshape(B, S, DOUT)


def kernel(x, base, mask, coeff):
    nc = _get_nc()
    in_maps = make_in_maps(np.asarray(x), np.asarray(base),
                           np.asarray(mask), np.asarray(coeff))
    res = run_bass_kernel_spmd(nc, in_maps, core_ids=list(range(8)))
    return assemble_out(res.results)


# revision 26
# speedup vs baseline: 1.0352x; 1.0256x over previous
"""BinaryDiff kernel for 8 TRN2 NeuronCores.

Computes out = x @ base + coeff * (x @ (2*mask - 1)) for
x [4,2048,4096] f32, base [4096,4096] f32, mask [4096,4096] i32,
coeff [] f32 -> out [4,2048,4096] f32.

Key algebraic fusion: dense + coeff*binary = x @ (base + coeff*(2*mask-1)),
so we fuse the weights on-device (one elementwise pass over base/mask) and
run a SINGLE matmul in bf16 (fp32 PSUM accumulation).

Sharding (tensor-parallel 2x4 grid, no collectives):
  - rows (B*S = 8192) split 2 ways  -> 4096 rows/core
  - out cols (4096)   split 4 ways  -> 1024 cols/core
Each core fuses its own W shard [4096, 1024] (bf16, SBUF-resident),
streams its x shard block-by-block (128 rows), casts to bf16, transposes
k-tiles via the PE array, and accumulates [128, 512] PSUM tiles over K.

Pipeline shape (from trace analysis): the contraction K=4096 is split into
quarter-rounds whose PSUM partials are combined on the DVE; blocks run in
super-blocks of 8, every super-block streams x in round-aligned column
chunks (small live x^T set, transposes interleave between rounds), chunk
production runs one stage ahead of the matmuls, and during super-block 0
the W-fusion DMAs are woven 1:1 with the x chunks of the matching k range.
Measured ~555 us on silicon (PE issue-rate-limited; bf16 matmul roofline
for this shard is ~442 us + ~66 us of PE transposes).
"""

import numpy as np
from contextlib import ExitStack

import concourse.bass as bass
import concourse.mybir as mybir
import concourse.tile as tile
from concourse import bacc
from concourse.bass_utils import run_bass_kernel_spmd
from concourse.masks import make_identity

P = 128
B, S, DIN, DOUT = 4, 2048, 4096, 4096
P_ROWS, Q_COLS = 2, 4           # core grid: 2 row-shards x 4 col-shards
BS = B * S                      # 8192
BS_C = BS // P_ROWS             # 4096 rows per core
NO_C = DOUT // Q_COLS           # 1024 out cols per core
MM_N = 512                      # matmul moving free dim (1 PSUM bank of f32)
SB_G = 8                        # blocks per super-block

f32 = mybir.dt.float32
bf16 = mybir.dt.bfloat16
i32 = mybir.dt.int32


def emit_kernel(tc, x_ap, base_ap, mask_ap, coeff_ap, out_ap,
                bs_c, din, no_c):
    """Emit the per-core Tile program. Shapes parameterized for sim tests."""
    nc = tc.nc
    kt_n = din // P            # k tiles
    nblk = bs_c // P           # 128-row x blocks
    ot_n = no_c // MM_N        # psum-group column tiles per block
    half = din // 2            # x row-block loaded as two DMAs (steady state)
    n_rounds = 4 if kt_n % 4 == 0 else 2
    kq = kt_n // n_rounds      # k tiles per round
    rounds = [(r * kq, (r + 1) * kq) for r in range(n_rounds)]
    sw = min(4, kq)           # k-tiles per x^T strip

    with ExitStack() as ctx:
        const = ctx.enter_context(tc.tile_pool(name="const", bufs=1))
        wpool = ctx.enter_context(tc.tile_pool(name="wpool", bufs=kt_n))
        fb = ctx.enter_context(tc.tile_pool(name="fbase", bufs=3))
        fm = ctx.enter_context(tc.tile_pool(name="fmask", bufs=3))
        fs = ctx.enter_context(tc.tile_pool(name="fsgn", bufs=2))
        xfp = ctx.enter_context(tc.tile_pool(name="xf", bufs=4))
        xbp = ctx.enter_context(tc.tile_pool(name="xb", bufs=5))
        xtp = ctx.enter_context(tc.tile_pool(name="xt", bufs=33))
        evp = ctx.enter_context(tc.tile_pool(name="ev", bufs=SB_G + 1))
        mmp = ctx.enter_context(tc.tile_pool(name="mmpsum", bufs=6, space="PSUM"))
        trp = ctx.enter_context(tc.tile_pool(name="trpsum", bufs=2, space="PSUM"))

        # --- constants: identity for PE transpose, coeff broadcast ---
        ident = const.tile([P, P], bf16)
        make_identity(nc, ident)

        c_sb = const.tile([1, 1], f32)
        nc.sync.dma_start(c_sb[:], coeff_ap[:])
        ones = const.tile([1, P], f32)
        nc.any.memset(ones[:], 1.0)
        cps = mmp.tile([P, MM_N], f32, tag="ps")
        # [128,1] = ones.T @ coeff : broadcasts the runtime scalar across partitions
        nc.tensor.matmul(cps[:, 0:1], ones[:], c_sb[:], start=True, stop=True)
        twoc = const.tile([P, 1], f32)
        negc = const.tile([P, 1], f32)
        nc.vector.tensor_scalar_mul(twoc[:], cps[:, 0:1], 2.0)
        nc.vector.tensor_scalar_mul(negc[:], cps[:, 0:1], -1.0)

        # --- W fusion: W[kt] = bf16(base + (2c)*mask - c), SBUF resident ---
        wtiles = [None] * kt_n

        def emit_fusion(kt):
            bt = fb.tile([P, no_c], f32)
            nc.sync.dma_start(bt[:], base_ap[kt * P:(kt + 1) * P, :])
            mt = fm.tile([P, no_c], i32)
            nc.sync.dma_start(mt[:], mask_ap[kt * P:(kt + 1) * P, :])
            sg = fs.tile([P, no_c], f32)
            nc.gpsimd.tensor_scalar(sg[:], mt[:], twoc[:], negc[:],
                                    mybir.AluOpType.mult, mybir.AluOpType.add)
            wt = wpool.tile([P, no_c], bf16)
            nc.vector.tensor_tensor(wt[:], sg[:], bt[:], mybir.AluOpType.add)
            wtiles[kt] = wt

        # --- per-block strip production: load, cast, PE-transpose ---
        # chunk = x columns [c0, c1); produces strips (one [128, 4*128]
        # x^T tile per 4 k-tiles) for those columns into `strips` dict.
        def emit_chunk(b, c0, c1, strips):
            row = slice(b * P, (b + 1) * P)
            xf = xfp.tile([P, c1 - c0], f32, tag="xf", name="xf")
            nc.sync.dma_start(xf[:], x_ap[row, c0:c1])
            xb = xbp.tile([P, c1 - c0], bf16, tag="xb", name="xb")
            nc.scalar.copy(xb[:], xf[:])
            for s in range((c1 - c0) // (sw * P)):
                pt = trp.tile([P, sw * P], bf16, tag="pt", name="pt")
                for j in range(sw):
                    col = s * sw * P + j * P
                    nc.tensor.transpose(pt[:, j * P:(j + 1) * P],
                                        xb[:, col:col + P], ident[:])
                st = xtp.tile([P, sw * P], bf16, tag="st", name="st")
                nc.vector.tensor_copy(st[:], pt[:])
                strips[(c0 + s * sw * P) // P] = st

        def emit_round(strips, ev, kt_lo, kt_hi, first):
            for ot in range(ot_n):
                ps = mmp.tile([P, MM_N], f32, tag="ps", name="ps")
                for kt in range(kt_lo, kt_hi):
                    st = strips[(kt // sw) * sw]
                    nc.tensor.matmul(
                        ps[:],
                        st[:, (kt % sw) * P:(kt % sw + 1) * P],
                        wtiles[kt][:, ot * MM_N:(ot + 1) * MM_N],
                        start=(kt == kt_lo), stop=(kt == kt_hi - 1),
                    )
                evs = ev[:, ot * MM_N:(ot + 1) * MM_N]
                if first:
                    nc.vector.tensor_copy(evs, ps[:])
                else:
                    nc.vector.tensor_tensor(evs, evs, ps[:],
                                            mybir.AluOpType.add)

        # --- main schedule ---
        # Unified (block, k-round) staging with one-stage lookahead: the x
        # chunks (DMA + cast + PE-transpose) for stage s+1 are emitted before
        # the matmul rounds of stage s, so strip production overlaps the
        # previous round's matmuls and round boundaries have no PE->DVE->PE
        # handoff bubble. During super-block 0 the W-fusion DMAs are woven
        # 1:1 with the x chunks of the matching k range.
        fused = [False] * kt_n
        strips_of = {}
        ev_of = {}
        # Super-block 0 runs twice-as-fine k-rounds (matching the W-fusion
        # arrival granularity); steady super-blocks use the coarser rounds.
        rounds0 = rounds
        stages = []
        for sb0 in range(0, nblk, SB_G):
            blocks = list(range(sb0, min(sb0 + SB_G, nblk)))
            rl = rounds0 if sb0 == 0 else rounds
            for ri, (klo, khi) in enumerate(rl):
                stages.append((blocks, klo, khi, ri == 0, ri == len(rl) - 1))

        def emit_stage_chunks(blocks, klo, khi):
            kts_todo = [kt for kt in range(klo, khi) if not fused[kt]]
            nb = len(blocks)
            for i, b in enumerate(blocks):
                if b not in ev_of:
                    strips_of[b] = {}
                    ev_of[b] = evp.tile([P, no_c], f32, tag="ev", name="ev")
                emit_chunk(b, klo * P, khi * P, strips_of[b])
                n0 = len(kts_todo) * i // nb
                n1 = len(kts_todo) * (i + 1) // nb
                for kt in kts_todo[n0:n1]:
                    emit_fusion(kt)
                    fused[kt] = True

        emit_stage_chunks(*stages[0][:3])
        for si, (blocks, klo, khi, first, last) in enumerate(stages):
            if si + 1 < len(stages):
                emit_stage_chunks(*stages[si + 1][:3])
            for b in blocks:
                emit_round(strips_of[b], ev_of[b], klo, khi, first=first)
                if last:
                    row = slice(b * P, (b + 1) * P)
                    nc.sync.dma_start(out_ap[row, :], ev_of[b][:])
            if last:
                for b in blocks:
                    del strips_of[b], ev_of[b]


def build_nc(bs_c=BS_C, din=DIN, no_c=NO_C):
    nc = bacc.Bacc("TRN2", target_bir_lowering=False, debug=False, num_devices=8)
    x_ap = nc.dram_tensor("x", [bs_c, din], f32, kind="ExternalInput").ap()
    base_ap = nc.dram_tensor("base", [din, no_c], f32, kind="ExternalInput").ap()
    mask_ap = nc.dram_tensor("mask", [din, no_c], i32, kind="ExternalInput").ap()
    coeff_ap = nc.dram_tensor("coeff", [1, 1], f32, kind="ExternalInput").ap()
    out_ap = nc.dram_tensor("out", [bs_c, no_c], f32, kind="ExternalOutput").ap()
    with tile.TileContext(nc) as tc:
        emit_kernel(tc, x_ap, base_ap, mask_ap, coeff_ap, out_ap,
                    bs_c, din, no_c)
    nc.compile()
    return nc


_NC_CACHE = {}


def _get_nc():
    if "nc" not in _NC_CACHE:
        _NC_CACHE["nc"] = build_nc()
    return _NC_CACHE["nc"]


def make_in_maps(x, base, mask, coeff):
    """Shard full inputs across the 2x4 core grid (cores 0..7)."""
    xf = np.ascontiguousarray(x.reshape(BS, DIN).astype(np.float32, copy=False))
    coeff2d = np.asarray(coeff, dtype=np.float32).reshape(1, 1)
    in_maps = []
    for cid in range(8):
        pi, qi = divmod(cid, Q_COLS)
        in_maps.append({
            "x": np.ascontiguousarray(xf[pi * BS_C:(pi + 1) * BS_C, :]),
            "base": np.ascontiguousarray(
                base[:, qi * NO_C:(qi + 1) * NO_C].astype(np.float32, copy=False)),
            "mask": np.ascontiguousarray(
                mask[:, qi * NO_C:(qi + 1) * NO_C].astype(np.int32, copy=False)),
            "coeff": coeff2d,
        })
    return in_maps


def assemble_out(results):
    out = np.empty((BS, DOUT), dtype=np.float32)
    for cid in range(8):
        pi, qi = divmod(cid, Q_COLS)
        out[pi * BS_C:(pi + 1) * BS_C, qi * NO_C:(qi + 1) * NO_C] = \
            results[cid]["out"]
    return out.reshape(B, S, DOUT)


def kernel(x, base, mask, coeff):
    nc = _get_nc()
    in_maps = make_in_maps(np.asarray(x), np.asarray(base),
                           np.asarray(mask), np.asarray(coeff))
    res = run_bass_kernel_spmd(nc, in_maps, core_ids=list(range(8)))
    return assemble_out(res.results)
